# revision 46
# baseline (speedup 1.0000x reference)
"""HSTU positional encoder on Trainium2, SPMD across 8 NeuronCores.

out[t] = seq_embeddings[t] * sqrt(D) + pos_weight[pos[t]]

pos[t] is derived from the ragged sequence structure (seq_offsets /
seq_lengths) on the host (tiny int metadata); the heavy memory work
(embedding read, table-row gather, output write) runs on device. Tokens
are split evenly across the 8 cores.

dtype strategy: the harness gate is rel_err < 2e-2 (max-abs over
max-abs), so narrow dtypes carry the traffic. The host folds alpha into
an fp16 cast of emb, pre-scales the table by 512 into fp8e4m3's normal
range, and upcasts the device's fp16 output back to fp32. Measured
rel_err ~3.4e-4 (dominated by fp16 rounding). Per-core HBM traffic
drops 48MB -> 21MB (emb 8.4 + gathered rows 4.2 + out 8.4).

Device kernel ("runs" layout): partition p owns the 64 consecutive
tokens [p*64, (p+1)*64) of the core shard; iteration i covers token
slice [offs[i], offs[i]+CS[i]) of every partition. Within one sequence
pos descends by exactly 1 per token, so a block of C consecutive tokens
needs C contiguous table rows; one indirect-DMA index then moves C*D
elements per descriptor (SWDGE descriptor generation is the gather's
rate limiter, so descriptors are kept big and calls few). The token
reversal (block rows ascend while tokens descend) is folded into the
DVE input access pattern via a negative stride. Compute is a single DVE
scalar_tensor_tensor per iteration: out = g*(1/512) + e.

Issue order is phase-sorted with one SBUF buffer per iteration (no
reuse): all emb loads enqueue on the sync HWDGE ring immediately,
gathers stream on the SWDGE queue once idx lands, STTs fire as operands
arrive, stores drain on the scalar HWDGE ring. The CS taper (small
iterations first) makes the first store fire early; the small last
iteration shortens the drain.

Blocks that are not a clean descending run (sequence boundaries or
clipping; a handful of blocks per core) are excluded on host: the
device result for those tokens is ignored and the host recomputes them
with identical arithmetic during the unshard/overlay step (<0.5% of
tokens, no device serialization).
"""

import ml_dtypes
import numpy as np

import concourse.bacc as bacc
import concourse.bass as bass
import concourse.mybir as mybir
import concourse.tile as tile
from concourse.bass_utils import run_bass_kernel_spmd

N_CORES = 8
TOTAL = 65536
D = 512
TABLE_ROWS = 8192
PART = 128
TOK_PER_CORE = TOTAL // N_CORES      # 8192
TILES = TOK_PER_CORE // PART         # 64 tokens per partition
ALPHA = float(np.sqrt(D))

# tunables
LAYOUT = "runs"   # "runs": run-block gather; "tok": row-per-token gather
CS = [4, 8, 16, 16, 8, 8, 4]  # per-iteration run lengths (sum TILES)
K = 4             # ("tok" layout) token-tiles per compute iteration
BUFS = 4          # ("tok" layout) tile-pool buffering depth
EMB_DT = "fp16"   # device emb dtype (host sends emb*sqrt(D) in this dtype)
TAB_DT = "fp8"    # device table dtype (host sends table*TAB_SCALE)
OUT_DT = "fp16"   # device out dtype (host upcasts to f32)
TAB_SCALE = 512.0

_DT = {"f32": (mybir.dt.float32, np.float32),
       "bf16": (mybir.dt.bfloat16, ml_dtypes.bfloat16),
       "fp16": (mybir.dt.float16, np.float16),
       "fp8": (mybir.dt.float8e4, ml_dtypes.float8_e4m3)}

_cache: dict = {}


def _build_nc():
    """Fallback "tok" layout: one gathered table row per token."""
    iters = TILES // K
    emb_dt = _DT[EMB_DT][0]
    tab_dt = _DT[TAB_DT][0]
    out_dt = _DT[OUT_DT][0]
    nc = bacc.Bacc("TRN2", target_bir_lowering=False, debug=False)
    emb = nc.dram_tensor("emb", [TOK_PER_CORE, D], emb_dt,
                         kind="ExternalInput")
    idx = nc.dram_tensor("idx", [PART, TILES], mybir.dt.int32,
                         kind="ExternalInput")
    table = nc.dram_tensor("table", [TABLE_ROWS, D], tab_dt,
                           kind="ExternalInput")
    out = nc.dram_tensor("out", [TOK_PER_CORE, D], out_dt,
                         kind="ExternalOutput")

    # iteration i, SBUF column block k, partition p <-> token (i*K+k)*128+p
    emb_v = emb.ap().rearrange("(n k p) d -> n p k d", k=K, p=PART)
    out_v = out.ap().rearrange("(n k p) d -> n p k d", k=K, p=PART)

    with tile.TileContext(nc) as tc:
        with (
            tc.tile_pool(name="idxp", bufs=1) as idxp,
            tc.tile_pool(name="sbuf", bufs=BUFS) as pool,
        ):
            idx_sb = idxp.tile([PART, TILES], mybir.dt.int32)
            nc.sync.dma_start(idx_sb[:], idx.ap())
            for i in range(iters):
                e = pool.tile([PART, K * D], emb_dt, tag="emb")
                nc.sync.dma_start(
                    e[:].rearrange("p (k d) -> p k d", k=K), emb_v[i])
                o = pool.tile([PART, K * D], out_dt, tag="out")
                g = pool.tile([PART, K * D], tab_dt, tag="gat")
                for k in range(K):
                    nc.gpsimd.indirect_dma_start(
                        out=g[:, k * D:(k + 1) * D],
                        out_offset=None,
                        in_=table.ap(),
                        in_offset=bass.IndirectOffsetOnAxis(
                            ap=idx_sb[:, i * K + k:i * K + k + 1], axis=0),
                    )
                nc.vector.scalar_tensor_tensor(
                    o[:], g[:], 1.0 / TAB_SCALE, e[:],
                    op0=mybir.AluOpType.mult,
                    op1=mybir.AluOpType.add)
                nc.scalar.dma_start(
                    out_v[i], o[:].rearrange("p (k d) -> p k d", k=K))
    nc.compile()
    return nc


def _build_nc_runs():
    CS_ = list(CS)
    assert sum(CS_) == TILES
    iters = len(CS_)
    offs = [0]
    for c in CS_:
        offs.append(offs[-1] + c)
    emb_dt = _DT[EMB_DT][0]
    tab_dt = _DT[TAB_DT][0]
    out_dt = _DT[OUT_DT][0]
    nc = bacc.Bacc("TRN2", target_bir_lowering=False, debug=False)
    emb = nc.dram_tensor("emb", [TOK_PER_CORE, D], emb_dt,
                         kind="ExternalInput")
    idx = nc.dram_tensor("idx", [PART, iters], mybir.dt.int32,
                         kind="ExternalInput")
    table = nc.dram_tensor("table", [TABLE_ROWS, D], tab_dt,
                           kind="ExternalInput")
    out = nc.dram_tensor("out", [TOK_PER_CORE, D], out_dt,
                         kind="ExternalOutput")

    # token (core-local) = p*64 + offs[i] + c
    emb_b = emb.ap()
    out_b = out.ap()

    def dram_view(base, i):
        return bass.AP(base.tensor, base.offset + offs[i] * D,
                       [[TILES * D, PART], [D, CS_[i]], [1, D]])

    with tile.TileContext(nc) as tc:
        with (
            tc.tile_pool(name="idxp", bufs=1) as idxp,
            tc.tile_pool(name="sbuf", bufs=1) as pool,
        ):
            # warm up the SWDGE path before idx arrives so the first real
            # gather pays no kickoff latency
            widx = idxp.tile([2, 1], mybir.dt.int32, tag="widx")
            nc.gpsimd.memset(widx[:], 0)
            warm = idxp.tile([2, D], _DT[TAB_DT][0], tag="warm")
            nc.gpsimd.indirect_dma_start(
                out=warm[:], out_offset=None, in_=table.ap(),
                in_offset=bass.IndirectOffsetOnAxis(ap=widx[:, :1], axis=0),
            )

            idx_sb = idxp.tile([PART, iters], mybir.dt.int32)
            nc.sync.dma_start(idx_sb[:], idx.ap())

            e_t = [pool.tile([PART, CS_[i] * D], emb_dt, tag=f"emb{i}",
                             name=f"e{i}") for i in range(iters)]
            g_t = [pool.tile([PART, CS_[i] * D], tab_dt, tag=f"gat{i}",
                             name=f"g{i}") for i in range(iters)]
            o_t = [pool.tile([PART, CS_[i] * D], out_dt, tag=f"out{i}",
                             name=f"o{i}") for i in range(iters)]

            for i in range(iters):
                nc.sync.dma_start(
                    e_t[i][:].rearrange("p (c d) -> p c d", c=CS_[i]),
                    dram_view(emb_b, i))
            for i in range(iters):
                nc.gpsimd.indirect_dma_start(
                    out=g_t[i][:],
                    out_offset=None,
                    in_=table.ap(),
                    in_offset=bass.IndirectOffsetOnAxis(
                        ap=idx_sb[:, i:i + 1], axis=0),
                )
            for i in range(iters):
                C = CS_[i]
                # run base holds rows ascending = tokens reversed; read g
                # with a reversed c-axis AP to undo it
                g3 = g_t[i][:].rearrange("p (c d) -> p c d", c=C)
                g_rev = bass.AP(
                    g3.tensor, g3.offset + (C - 1) * D,
                    [g3.ap[0], [-D, C], [1, D]])
                nc.vector.scalar_tensor_tensor(
                    o_t[i][:].rearrange("p (c d) -> p c d", c=C),
                    g_rev, 1.0 / TAB_SCALE,
                    e_t[i][:].rearrange("p (c d) -> p c d", c=C),
                    op0=mybir.AluOpType.mult,
                    op1=mybir.AluOpType.add)
            for i in range(iters):
                st_eng = (nc.sync, nc.scalar)[i % 2]
                st_eng.dma_start(
                    dram_view(out_b, i),
                    o_t[i][:].rearrange("p (c d) -> p c d", c=CS_[i]))

    nc.compile()
    return nc


def _get_nc():
    key = ("nc", LAYOUT, tuple(CS), K, BUFS,
           EMB_DT, TAB_DT, OUT_DT, TAB_SCALE)
    if key not in _cache:
        _cache[key] = _build_nc_runs() if LAYOUT == "runs" else _build_nc()
    return _cache[key]


def _pos_indices(seq_lengths, seq_offsets, total):
    offsets = np.asarray(seq_offsets).astype(np.int64)
    lens = np.asarray(seq_lengths).astype(np.int64)
    tok = np.arange(total, dtype=np.int64)
    seg = np.searchsorted(offsets, tok, side="right") - 1
    high = np.minimum(lens, TABLE_ROWS - 1)
    pos = high[seg] - (tok - offsets[seg])
    return np.clip(pos, 0, TABLE_ROWS - 1).astype(np.int32)


def _core_inputs(c, emb, table, pos):
    sl = slice(c * TOK_PER_CORE, (c + 1) * TOK_PER_CORE)
    if LAYOUT == "tok":
        idx_t = np.ascontiguousarray(pos[sl].reshape(TILES, PART).T)
        return {"emb": emb[sl], "idx": idx_t, "table": table}, None
    CS_ = list(CS)
    iters = len(CS_)
    offs = np.concatenate([[0], np.cumsum(CS_)]).astype(int)
    pos_c = pos[sl]
    pm = pos_c.reshape(PART, TILES).astype(np.int64)
    idx_arr = np.empty((PART, iters), np.int32)
    tok_list = []
    for j, C in enumerate(CS_):
        blk = pm[:, offs[j]:offs[j] + C]
        first = blk[:, 0]
        corrupt = (blk != first[:, None] - np.arange(C)).any(axis=1)
        idx_arr[:, j] = np.clip(first - (C - 1), 0, TABLE_ROWS - C)
        pp = np.nonzero(corrupt)[0]
        if len(pp):
            tok_list.append(
                ((pp * TILES + offs[j])[:, None] + np.arange(C)).ravel())
    toks = (np.concatenate(tok_list) if tok_list
            else np.empty(0, np.int64))
    idx_arr = np.ascontiguousarray(idx_arr)
    return {"emb": emb[sl], "idx": idx_arr, "table": table}, toks


def _run(max_seq_len, seq_lengths, seq_offsets, seq_embeddings, pos_weight,
         trace=False):
    emb_f32 = np.asarray(seq_embeddings, dtype=np.float32)
    tab_f32 = np.asarray(pos_weight, dtype=np.float32)
    emb = np.ascontiguousarray((emb_f32 * ALPHA).astype(_DT[EMB_DT][1]))
    table = np.ascontiguousarray(
        (tab_f32 * TAB_SCALE).astype(_DT[TAB_DT][1]))
    pos = _pos_indices(seq_lengths, seq_offsets, emb.shape[0])
    packs = [_core_inputs(c, emb, table, pos) for c in range(N_CORES)]
    in_maps = [p[0] for p in packs]
    toks_per_core = [p[1] for p in packs]
    res = run_bass_kernel_spmd(_get_nc(), in_maps, list(range(N_CORES)),
                               trace=trace)
    full = np.concatenate([res.results[c]["out"] for c in range(N_CORES)],
                          axis=0).astype(np.float32)
    for c, toks in enumerate(toks_per_core):
        if toks is not None and len(toks):
            g = toks + c * TOK_PER_CORE
            v = (emb[g].astype(np.float32)
                 + table[pos[g]].astype(np.float32) / TAB_SCALE)
            full[g] = v.astype(_DT[OUT_DT][1]).astype(np.float32)
    return full, res


def kernel(max_seq_len, seq_lengths, seq_offsets, seq_embeddings, pos_weight):
    full, _ = _run(max_seq_len, seq_lengths, seq_offsets, seq_embeddings,
                   pos_weight)
    return full


# revision 49
# speedup vs baseline: 1.0181x; 1.0181x over previous
"""HSTU positional encoder on Trainium2, SPMD across 8 NeuronCores.

out[t] = seq_embeddings[t] * sqrt(D) + pos_weight[pos[t]]

pos[t] is derived from the ragged sequence structure (seq_offsets /
seq_lengths) on the host (tiny int metadata); the heavy memory work
(embedding read, table-row gather, output write) runs on device. Tokens
are split evenly across the 8 cores.

dtype strategy: the harness gate is rel_err < 2e-2 (max-abs over
max-abs), so narrow dtypes carry the traffic. The host folds alpha into
an fp16 cast of emb, pre-scales the table by 512 into fp8e4m3's normal
range, and upcasts the device's fp16 output back to fp32. Measured
rel_err ~3.4e-4 (dominated by fp16 rounding). Per-core HBM traffic
drops 48MB -> 21MB (emb 8.4 + gathered rows 4.2 + out 8.4).

Device kernel ("runs" layout): partition p owns the 64 consecutive
tokens [p*64, (p+1)*64) of the core shard; iteration i covers token
slice [offs[i], offs[i]+CS[i]) of every partition. Within one sequence
pos descends by exactly 1 per token, so a block of C consecutive tokens
needs C contiguous table rows; one indirect-DMA index then moves C*D
elements per descriptor (SWDGE descriptor generation is the gather's
rate limiter, so descriptors are kept big and calls few). The token
reversal (block rows ascend while tokens descend) is folded into the
DVE input access pattern via a negative stride. Compute is a single DVE
scalar_tensor_tensor per iteration: out = g*(1/512) + e.

Issue order is phase-sorted with one SBUF buffer per iteration (no
reuse): all emb loads enqueue on the sync HWDGE ring immediately,
gathers stream on the SWDGE queue once idx lands, STTs fire as operands
arrive, stores drain on the scalar HWDGE ring. The CS taper (small
iterations first) makes the first store fire early; the small last
iteration shortens the drain.

Blocks that are not a clean descending run (sequence boundaries or
clipping; a handful of blocks per core) are excluded on host: the
device result for those tokens is ignored and the host recomputes them
with identical arithmetic during the unshard/overlay step (<0.5% of
tokens, no device serialization).
"""

import ml_dtypes
import numpy as np

import concourse.bacc as bacc
import concourse.bass as bass
import concourse.mybir as mybir
import concourse.tile as tile
from concourse.bass_utils import run_bass_kernel_spmd

N_CORES = 8
TOTAL = 65536
D = 512
TABLE_ROWS = 8192
PART = 128
TOK_PER_CORE = TOTAL // N_CORES      # 8192
TILES = TOK_PER_CORE // PART         # 64 tokens per partition
ALPHA = float(np.sqrt(D))

# tunables
LAYOUT = "runs"   # "runs": run-block gather; "tok": row-per-token gather
CS = [4, 8, 16, 16, 8, 8, 4]  # per-iteration run lengths (sum TILES)
K = 4             # ("tok" layout) token-tiles per compute iteration
BUFS = 4          # ("tok" layout) tile-pool buffering depth
EMB_DT = "fp16"   # device emb dtype (host sends emb*sqrt(D) in this dtype)
TAB_DT = "fp8"    # device table dtype (host sends table*TAB_SCALE)
OUT_DT = "fp16"   # device out dtype (host upcasts to f32)
TAB_SCALE = 512.0

_DT = {"f32": (mybir.dt.float32, np.float32),
       "bf16": (mybir.dt.bfloat16, ml_dtypes.bfloat16),
       "fp16": (mybir.dt.float16, np.float16),
       "fp8": (mybir.dt.float8e4, ml_dtypes.float8_e4m3)}

_cache: dict = {}


def _build_nc():
    """Fallback "tok" layout: one gathered table row per token."""
    iters = TILES // K
    emb_dt = _DT[EMB_DT][0]
    tab_dt = _DT[TAB_DT][0]
    out_dt = _DT[OUT_DT][0]
    nc = bacc.Bacc("TRN2", target_bir_lowering=False, debug=False)
    emb = nc.dram_tensor("emb", [TOK_PER_CORE, D], emb_dt,
                         kind="ExternalInput")
    idx = nc.dram_tensor("idx", [PART, TILES], mybir.dt.int32,
                         kind="ExternalInput")
    table = nc.dram_tensor("table", [TABLE_ROWS, D], tab_dt,
                           kind="ExternalInput")
    out = nc.dram_tensor("out", [TOK_PER_CORE, D], out_dt,
                         kind="ExternalOutput")

    # iteration i, SBUF column block k, partition p <-> token (i*K+k)*128+p
    emb_v = emb.ap().rearrange("(n k p) d -> n p k d", k=K, p=PART)
    out_v = out.ap().rearrange("(n k p) d -> n p k d", k=K, p=PART)

    with tile.TileContext(nc) as tc:
        with (
            tc.tile_pool(name="idxp", bufs=1) as idxp,
            tc.tile_pool(name="sbuf", bufs=BUFS) as pool,
        ):
            idx_sb = idxp.tile([PART, TILES], mybir.dt.int32)
            nc.sync.dma_start(idx_sb[:], idx.ap())
            for i in range(iters):
                e = pool.tile([PART, K * D], emb_dt, tag="emb")
                nc.sync.dma_start(
                    e[:].rearrange("p (k d) -> p k d", k=K), emb_v[i])
                o = pool.tile([PART, K * D], out_dt, tag="out")
                g = pool.tile([PART, K * D], tab_dt, tag="gat")
                for k in range(K):
                    nc.gpsimd.indirect_dma_start(
                        out=g[:, k * D:(k + 1) * D],
                        out_offset=None,
                        in_=table.ap(),
                        in_offset=bass.IndirectOffsetOnAxis(
                            ap=idx_sb[:, i * K + k:i * K + k + 1], axis=0),
                    )
                nc.vector.scalar_tensor_tensor(
                    o[:], g[:], 1.0 / TAB_SCALE, e[:],
                    op0=mybir.AluOpType.mult,
                    op1=mybir.AluOpType.add)
                nc.scalar.dma_start(
                    out_v[i], o[:].rearrange("p (k d) -> p k d", k=K))
    nc.compile()
    return nc


def _build_nc_runs():
    CS_ = list(CS)
    assert sum(CS_) == TILES
    iters = len(CS_)
    offs = [0]
    for c in CS_:
        offs.append(offs[-1] + c)
    emb_dt = _DT[EMB_DT][0]
    tab_dt = _DT[TAB_DT][0]
    out_dt = _DT[OUT_DT][0]
    nc = bacc.Bacc("TRN2", target_bir_lowering=False, debug=False)
    emb = nc.dram_tensor("emb", [TOK_PER_CORE, D], emb_dt,
                         kind="ExternalInput")
    idx = nc.dram_tensor("idx", [PART, iters], mybir.dt.int32,
                         kind="ExternalInput")
    table = nc.dram_tensor("table", [TABLE_ROWS, D], tab_dt,
                           kind="ExternalInput")
    out = nc.dram_tensor("out", [TOK_PER_CORE, D], out_dt,
                         kind="ExternalOutput")

    # token (core-local) = p*64 + offs[i] + c
    emb_b = emb.ap()
    out_b = out.ap()

    def dram_view(base, i):
        return bass.AP(base.tensor, base.offset + offs[i] * D,
                       [[TILES * D, PART], [D, CS_[i]], [1, D]])

    with tile.TileContext(nc) as tc:
        with (
            tc.tile_pool(name="idxp", bufs=1) as idxp,
            tc.tile_pool(name="sbuf", bufs=1) as pool,
        ):
            # warm up the SWDGE path before idx arrives so the first real
            # gather pays no kickoff latency
            widx = idxp.tile([2, 1], mybir.dt.int32, tag="widx")
            nc.gpsimd.memset(widx[:], 0)
            warm = idxp.tile([2, D], _DT[TAB_DT][0], tag="warm")
            nc.gpsimd.indirect_dma_start(
                out=warm[:], out_offset=None, in_=table.ap(),
                in_offset=bass.IndirectOffsetOnAxis(ap=widx[:, :1], axis=0),
            )

            idx_sb = idxp.tile([PART, iters], mybir.dt.int32)
            nc.sync.dma_start(idx_sb[:], idx.ap())

            e_t = [pool.tile([PART, CS_[i] * D], emb_dt, tag=f"emb{i}",
                             name=f"e{i}") for i in range(iters)]
            g_t = [pool.tile([PART, CS_[i] * D], tab_dt, tag=f"gat{i}",
                             name=f"g{i}") for i in range(iters)]
            o_t = [pool.tile([PART, CS_[i] * D], out_dt, tag=f"out{i}",
                             name=f"o{i}") for i in range(iters)]

            # loads and stores share the sync ring, interleaved in
            # production order (program order keeps producer STT before each
            # store; the ring sees L0..L2, L3, S0, L4, S1, ... so stores
            # ship as produced and no backlog forms at the end)
            AHEAD = 3

            def _store(i):
                nc.sync.dma_start(
                    dram_view(out_b, i),
                    o_t[i][:].rearrange("p (c d) -> p c d", c=CS_[i]))

            for i in range(iters):
                nc.sync.dma_start(
                    e_t[i][:].rearrange("p (c d) -> p c d", c=CS_[i]),
                    dram_view(emb_b, i))
                nc.gpsimd.indirect_dma_start(
                    out=g_t[i][:],
                    out_offset=None,
                    in_=table.ap(),
                    in_offset=bass.IndirectOffsetOnAxis(
                        ap=idx_sb[:, i:i + 1], axis=0),
                )
                C = CS_[i]
                # run base holds rows ascending = tokens reversed; read g
                # with a reversed c-axis AP to undo it
                g3 = g_t[i][:].rearrange("p (c d) -> p c d", c=C)
                g_rev = bass.AP(
                    g3.tensor, g3.offset + (C - 1) * D,
                    [g3.ap[0], [-D, C], [1, D]])
                nc.vector.scalar_tensor_tensor(
                    o_t[i][:].rearrange("p (c d) -> p c d", c=C),
                    g_rev, 1.0 / TAB_SCALE,
                    e_t[i][:].rearrange("p (c d) -> p c d", c=C),
                    op0=mybir.AluOpType.mult,
                    op1=mybir.AluOpType.add)
                if i >= AHEAD:
                    _store(i - AHEAD)
            for i in range(iters - AHEAD, iters):
                _store(i)

    nc.compile()
    return nc


def _get_nc():
    key = ("nc", LAYOUT, tuple(CS), K, BUFS,
           EMB_DT, TAB_DT, OUT_DT, TAB_SCALE)
    if key not in _cache:
        _cache[key] = _build_nc_runs() if LAYOUT == "runs" else _build_nc()
    return _cache[key]


def _pos_indices(seq_lengths, seq_offsets, total):
    offsets = np.asarray(seq_offsets).astype(np.int64)
    lens = np.asarray(seq_lengths).astype(np.int64)
    tok = np.arange(total, dtype=np.int64)
    seg = np.searchsorted(offsets, tok, side="right") - 1
    high = np.minimum(lens, TABLE_ROWS - 1)
    pos = high[seg] - (tok - offsets[seg])
    return np.clip(pos, 0, TABLE_ROWS - 1).astype(np.int32)


def _core_inputs(c, emb, table, pos):
    sl = slice(c * TOK_PER_CORE, (c + 1) * TOK_PER_CORE)
    if LAYOUT == "tok":
        idx_t = np.ascontiguousarray(pos[sl].reshape(TILES, PART).T)
        return {"emb": emb[sl], "idx": idx_t, "table": table}, None
    CS_ = list(CS)
    iters = len(CS_)
    offs = np.concatenate([[0], np.cumsum(CS_)]).astype(int)
    pos_c = pos[sl]
    pm = pos_c.reshape(PART, TILES).astype(np.int64)
    idx_arr = np.empty((PART, iters), np.int32)
    tok_list = []
    for j, C in enumerate(CS_):
        blk = pm[:, offs[j]:offs[j] + C]
        first = blk[:, 0]
        corrupt = (blk != first[:, None] - np.arange(C)).any(axis=1)
        idx_arr[:, j] = np.clip(first - (C - 1), 0, TABLE_ROWS - C)
        pp = np.nonzero(corrupt)[0]
        if len(pp):
            tok_list.append(
                ((pp * TILES + offs[j])[:, None] + np.arange(C)).ravel())
    toks = (np.concatenate(tok_list) if tok_list
            else np.empty(0, np.int64))
    idx_arr = np.ascontiguousarray(idx_arr)
    return {"emb": emb[sl], "idx": idx_arr, "table": table}, toks


def _run(max_seq_len, seq_lengths, seq_offsets, seq_embeddings, pos_weight,
         trace=False):
    emb_f32 = np.asarray(seq_embeddings, dtype=np.float32)
    tab_f32 = np.asarray(pos_weight, dtype=np.float32)
    emb = np.ascontiguousarray((emb_f32 * ALPHA).astype(_DT[EMB_DT][1]))
    table = np.ascontiguousarray(
        (tab_f32 * TAB_SCALE).astype(_DT[TAB_DT][1]))
    pos = _pos_indices(seq_lengths, seq_offsets, emb.shape[0])
    packs = [_core_inputs(c, emb, table, pos) for c in range(N_CORES)]
    in_maps = [p[0] for p in packs]
    toks_per_core = [p[1] for p in packs]
    res = run_bass_kernel_spmd(_get_nc(), in_maps, list(range(N_CORES)),
                               trace=trace)
    full = np.concatenate([res.results[c]["out"] for c in range(N_CORES)],
                          axis=0).astype(np.float32)
    for c, toks in enumerate(toks_per_core):
        if toks is not None and len(toks):
            g = toks + c * TOK_PER_CORE
            v = (emb[g].astype(np.float32)
                 + table[pos[g]].astype(np.float32) / TAB_SCALE)
            full[g] = v.astype(_DT[OUT_DT][1]).astype(np.float32)
    return full, res


def kernel(max_seq_len, seq_lengths, seq_offsets, seq_embeddings, pos_weight):
    full, _ = _run(max_seq_len, seq_lengths, seq_offsets, seq_embeddings,
                   pos_weight)
    return full
